# revision 1
# baseline (speedup 1.0000x reference)
"""Trainium2 Bass kernel for 16-head MHA (B=2, S=2048, D=1024, E=64).

Sharding: 8 cores = 2 batches x 4 head-groups. Each core computes 4 heads
(2 pairs of 2) for one batch and returns a partial output [2048, 1024]
(sum of its 4 heads' contributions after the output projection). Host sums
the 4 partials per batch.

Per-core pipeline (all matmuls on PE, fp32 PSUM accumulation):
  - projections QT/KT (feature-major, fp16 in, fp32r out), V (fp16 in,
    transposed on PE to token-major fp32r [V|1] tiles)
  - S^T = K Q^T per head pair, two heads row-packed in the 128x128 array
  - A^T = exp(S^T) on ACT (scale folded into W_query on host), fp32r
  - O^T accumulation with fused row-sum via the [V|1] ones column
  - softmax normalization: DVE reciprocal + GPSIMD partition-broadcast +
    DVE multiply (writes fp16 O^T)
  - output projection (fp16) accumulating both pairs, DMA out token-major
"""

import sys

sys.path.insert(0, "/opt/trn_rl_repo")

import numpy as np

import concourse.bass as bass
import concourse.bacc as bacc
import concourse.mybir as mybir
from concourse import tile
from concourse.tile_rust import add_dep_helper
from concourse.bass_interp import get_hw_module
from concourse.bass_utils import run_bass_kernel_spmd

F16 = mybir.dt.float16
F32 = mybir.dt.float32
F32R = mybir.dt.float32r
BF16 = mybir.dt.bfloat16

N_CORES = 8
T = 2048          # tokens per core (one batch)
D = 1024          # model dim
E = 64            # head dim
QC = 512          # query chunk
NQ = T // QC      # 4 query chunks
KB = 128          # key block
NKB = T // KB     # 16 key blocks
ND = D // 128     # 8 contraction chunks for projections

_CACHE = {}


def _build():
    nc = bacc.Bacc("TRN2", target_bir_lowering=False, debug=False,
                   num_devices=N_CORES)

    xqT = nc.dram_tensor("xqT", [D, T], F16, kind="ExternalInput").ap()
    xkT = nc.dram_tensor("xkT", [D, T], F16, kind="ExternalInput").ap()
    xvT = nc.dram_tensor("xvT", [D, T], F16, kind="ExternalInput").ap()
    # per-pair packed weights, layout [128, 8*128]: chunk d at cols d*128
    wq = [nc.dram_tensor(f"wq{p}", [128, D], F16, kind="ExternalInput").ap()
          for p in range(2)]
    wk = [nc.dram_tensor(f"wk{p}", [128, D], F16, kind="ExternalInput").ap()
          for p in range(2)]
    wv = [nc.dram_tensor(f"wv{p}", [128, D], F16, kind="ExternalInput").ap()
          for p in range(2)]
    wo = [nc.dram_tensor(f"wo{p}", [128, D], F16, kind="ExternalInput").ap()
          for p in range(2)]
    pout = nc.dram_tensor("pout", [T, D], F32, kind="ExternalOutput").ap()

    with tile.TileContext(nc) as tc:
        with (
            tc.tile_pool(name="consts", bufs=1) as consts,
            tc.tile_pool(name="persist", bufs=1) as persist,
            tc.tile_pool(name="xs", bufs=12) as xs,
            tc.tile_pool(name="at", bufs=3) as atp,
            tc.tile_pool(name="o2t", bufs=2) as o2tp,
            tc.tile_pool(name="os", bufs=3) as osp,
            tc.tile_pool(name="small", bufs=4) as smallp,
        ):
            # ---- constants ----
            wq_sb = [consts.tile([128, D], F16, tag=f"wq{p}", name=f"wq_sb{p}") for p in range(2)]
            wk_sb = [consts.tile([128, D], F16, tag=f"wk{p}", name=f"wk_sb{p}") for p in range(2)]
            wv_sb = [consts.tile([128, D], F16, tag=f"wv{p}", name=f"wv_sb{p}") for p in range(2)]
            wo_sb = [consts.tile([128, D], F16, tag=f"wo{p}", name=f"wo_sb{p}") for p in range(2)]
            for p in range(2):
                nc.sync.dma_start(wq_sb[p][:], wq[p][:])
                nc.sync.dma_start(wk_sb[p][:], wk[p][:])
                nc.sync.dma_start(wv_sb[p][:], wv[p][:])
                nc.sync.dma_start(wo_sb[p][:], wo[p][:])

            # ---- persistent activations ----
            # feature-major Q^T, K^T per pair: rows 0:64 head0, 64:128 head1
            qt = [[persist.tile([128, QC], F16, tag=f"qt{p}_{t}", name=f"qt{p}_{t}")
                   for t in range(NQ)] for p in range(2)]
            kt = [persist.tile([128, T], F16, tag=f"kt{p}", name=f"kt{p}") for p in range(2)]
            # token-major [V | 1] per (head, key-block): [128, 65] each
            v2 = [[persist.tile([128, 65], BF16, tag=f"v2_{h}_{b}", name=f"v2_{h}_{b}")
                   for b in range(NKB)] for h in range(4)]
            ones_f32 = consts.tile([128, 1], F32, tag="ones", name="ones_f32")
            nc.vector.memset(ones_f32[:], 1.0)
            for h in range(4):
                for b in range(NKB):
                    nc.vector.tensor_copy(v2[h][b][:, 64:65], ones_f32[:])

            # ---- phase 1: projections ----
            with (
                tc.tile_pool(name="psA", bufs=3, space="PSUM") as psA,
                tc.tile_pool(name="vtmp", bufs=1) as vtmp,
            ):
                def project(x_dram, w_sb, evac, pfx, post_p=None):
                    # d-outer so PE consumes each x chunk as its DMA lands;
                    # weights for chunk d reused across the 4 t-slices
                    xt = [None] * ND
                    for d in range(ND):
                        xt[d] = xs.tile([128, T], F16, tag="x", name=f"x{pfx}_{d}")
                        half = T // 2
                        nc.sync.dma_start(
                            xt[d][:, 0:half],
                            x_dram[d * 128:(d + 1) * 128, 0:half])
                        nc.sync.dma_start(
                            xt[d][:, half:T],
                            x_dram[d * 128:(d + 1) * 128, half:T])
                    for p in range(2):
                        pss = [psA.tile([128, QC], F32, tag="proj", name=f"proj{pfx}_{t}_{p}")
                               for t in range(NQ)]
                        for d in range(ND):
                            for t in range(NQ):
                                nc.tensor.matmul(
                                    pss[t][:], w_sb[p][:, d * 128:(d + 1) * 128],
                                    xt[d][:, t * QC:(t + 1) * QC],
                                    start=(d == 0), stop=(d == ND - 1))
                        for t in range(NQ):
                            evac(p, t, pss[t])
                        if post_p is not None:
                            post_p(p)

                def evac_kt(p, t, ps):
                    nc.scalar.activation(kt[p][:, t * QC:(t + 1) * QC],
                                         ps[:], mybir.ActivationFunctionType.Copy)

                def evac_qt(p, t, ps):
                    nc.scalar.activation(qt[p][t][:],
                                         ps[:], mybir.ActivationFunctionType.Copy)

                # K first, then V (+ transpose), then Q — lets attention start
                # as soon as possible while Q tiles still stream.
                project(xkT, wk_sb, evac_kt, "k")

                vt = [vtmp.tile([128, T], BF16, tag=f"vt{p}", name=f"vt{p}") for p in range(2)]

                def evac_vt(p, t, ps):
                    nc.vector.tensor_copy(vt[p][:, t * QC:(t + 1) * QC],
                                          ps[:])

                def transpose_v(p):
                    # token-major via DMA transpose (2-byte dtype), off the PE
                    for h in range(2):
                        for blk in range(NKB):
                            nc.sync.dma_start_transpose(
                                v2[2 * p + h][blk][:, 0:64],
                                vt[p][h * 64:(h + 1) * 64,
                                      blk * 128:(blk + 1) * 128])

                project(xvT, wv_sb, evac_vt, "v", post_p=transpose_v)

                project(xqT, wq_sb, evac_qt, "q")

            # ---- phase 2: attention + output projection ----
            with (
                tc.tile_pool(name="psS", bufs=2, space="PSUM") as psS,
                tc.tile_pool(name="psO", bufs=1, space="PSUM") as psO,
                tc.tile_pool(name="psP", bufs=2, space="PSUM") as psP,
            ):
                ost_live = {}

                def emit_outproj_group(qc, o2t, sub, oc, anchor):
                    q0 = qc * QC
                    if oc == 0:
                        ost_live[(qc, sub)] = osp.tile(
                            [128, D], F32, tag="os", name=f"os_{qc}_{sub}")
                    ost = ost_live[(qc, sub)]
                    pp = psP.tile([128, 512], F32, tag="pp", name=f"pp_{qc}_{sub}_{oc}")
                    for p in range(2):
                        mm = nc.tensor.matmul(
                            pp[:],
                            o2t[p][:, sub * 128:(sub + 1) * 128],
                            wo_sb[p][:, oc * 512:(oc + 1) * 512],
                            start=(p == 0), stop=(p == 1))
                        if p == 0 and anchor is not None:
                            add_dep_helper(mm.ins, anchor.ins, sync=False,
                                           reason="interleave outproj after S")
                    nc.vector.tensor_copy(
                        ost[:, oc * 512:(oc + 1) * 512], pp[:])
                    if oc == 1:
                        nc.sync.dma_start(
                            pout[q0 + sub * 128:q0 + (sub + 1) * 128, :],
                            ost[:])
                        del ost_live[(qc, sub)]

                def emit_outproj(qc, o2t, anchor=None):
                    for sub in range(4):
                        for oc in range(2):
                            emit_outproj_group(qc, o2t, sub, oc, anchor)

                pending = None
                for qc in range(NQ):
                    o2t = [o2tp.tile([128, QC], F16, tag=f"o2t{p}", name=f"o2t_{qc}_{p}")
                           for p in range(2)]
                    for p in range(2):
                        po = [psO.tile([65, QC], F32, tag=f"o{h}", name=f"po_{qc}_{p}_{h}")
                              for h in range(2)]
                        for kb in range(NKB):
                            k0 = kb * KB
                            ps = psS.tile([128, 2 * QC], F32, tag="s", name=f"s_{qc}_{p}_{kb}")
                            s_anchor = nc.tensor.matmul(
                                ps[:, 0:QC],
                                kt[p][0:64, k0:k0 + KB],
                                qt[p][qc][0:64, :],
                                start=True, stop=True, tile_position=(0, 0))
                            nc.tensor.matmul(
                                ps[:, QC:2 * QC],
                                kt[p][64:128, k0:k0 + KB],
                                qt[p][qc][64:128, :],
                                start=True, stop=True, tile_position=(64, 0))
                            at = atp.tile([128, 2 * QC], BF16, tag="at", name=f"at_{qc}_{p}_{kb}")
                            nc.scalar.activation(
                                at[:], ps[:], mybir.ActivationFunctionType.Exp)
                            for h in range(2):
                                nc.tensor.matmul(
                                    po[h][:],
                                    v2[2 * p + h][kb][:],
                                    at[:, h * QC:(h + 1) * QC],
                                    start=(kb == 0), stop=(kb == NKB - 1))
                            if p == 1 and pending is not None and kb % 2 == 1:
                                pqc, po2t = pending
                                emit_outproj_group(pqc, po2t, kb // 4,
                                                   (kb // 2) % 2, s_anchor)
                        # evacuate O^T fast (frees PSUM), then normalize
                        for h in range(2):
                            ot = smallp.tile([65, QC], F32, tag=f"ot{h}", name=f"ot_{qc}_{p}_{h}")
                            nc.vector.tensor_copy(ot[:], po[h][:])
                            r = smallp.tile([1, QC], F32, tag=f"r{h}", name=f"r_{qc}_{p}_{h}")
                            nc.vector.reciprocal(r[:], ot[64:65, :])
                            rb = smallp.tile([64, QC], F32, tag=f"rb{h}", name=f"rb_{qc}_{p}_{h}")
                            nc.gpsimd.partition_broadcast(rb[:], r[:])
                            nc.vector.tensor_mul(
                                o2t[p][h * 64:(h + 1) * 64, :],
                                ot[0:64, :], rb[:])
                        if p == 1:
                            pending = None
                    pending = (qc, o2t)
                emit_outproj(*pending)

    nc.compile()
    nc.m = get_hw_module(nc.m)
    return nc


def _pack_w(w_pair):
    # w_pair: [2, 1024, 64] -> [1024, 128] -> chunk-major [128, 8*128]
    w = np.concatenate([w_pair[0], w_pair[1]], axis=1)          # [1024, 128]
    return np.ascontiguousarray(
        w.reshape(ND, 128, 128).transpose(1, 0, 2).reshape(128, D))


def _pack_wo(wo_pair):
    # wo_pair: [2, 64, 1024] -> [128, 1024]
    return np.ascontiguousarray(np.concatenate([wo_pair[0], wo_pair[1]], axis=0))


def kernel(q, k, v, W_query, W_key, W_val, W_out, _trace=False):
    q = np.asarray(q, dtype=np.float32)
    k = np.asarray(k, dtype=np.float32)
    v = np.asarray(v, dtype=np.float32)
    W_query = np.asarray(W_query, dtype=np.float32)
    W_key = np.asarray(W_key, dtype=np.float32)
    W_val = np.asarray(W_val, dtype=np.float32)
    W_out = np.asarray(W_out, dtype=np.float32)

    if "nc" not in _CACHE:
        _CACHE["nc"] = _build()
    nc = _CACHE["nc"]

    norm = 1.0 / np.sqrt(E)
    xT = {}
    for b in range(2):
        xT[("q", b)] = np.ascontiguousarray(q[b].T).astype(np.float16)
        xT[("k", b)] = np.ascontiguousarray(k[b].T).astype(np.float16)
        xT[("v", b)] = np.ascontiguousarray(v[b].T).astype(np.float16)

    in_maps = []
    for c in range(N_CORES):
        b, g = c // 4, c % 4
        hs = [4 * g, 4 * g + 1, 4 * g + 2, 4 * g + 3]
        m = {
            "xqT": xT[("q", b)], "xkT": xT[("k", b)], "xvT": xT[("v", b)],
        }
        for p in range(2):
            hp = hs[2 * p:2 * p + 2]
            m[f"wq{p}"] = _pack_w(W_query[hp] * norm).astype(np.float16)
            m[f"wk{p}"] = _pack_w(W_key[hp]).astype(np.float16)
            m[f"wv{p}"] = _pack_w(W_val[hp]).astype(np.float16)
            m[f"wo{p}"] = _pack_wo(W_out[hp]).astype(np.float16)
        in_maps.append(m)

    res = run_bass_kernel_spmd(nc, in_maps, list(range(N_CORES)),
                               trace=_trace)
    parts = [res.results[c]["pout"] for c in range(N_CORES)]
    out = np.stack([
        parts[0] + parts[1] + parts[2] + parts[3],
        parts[4] + parts[5] + parts[6] + parts[7],
    ]).astype(np.float32)
    if _trace:
        _CACHE["last_result"] = res
    return out



# revision 3
# speedup vs baseline: 1.0907x; 1.0907x over previous
"""Trainium2 Bass kernel for 16-head MHA (B=2, S=2048, D=1024, E=64).

Sharding: 8 cores = 2 batches x 4 head-groups. Each core computes 4 heads
(2 pairs of 2) for one batch and returns a partial output [2048, 1024]
(sum of its 4 heads' contributions after the output projection) in fp16.
Host sums the 4 partials per batch.

Per-core schedule (software-pipelined so the ACT engine, which owns the
16.8M-element exp, is saturated from ~12us):
  - K projection, then Q chunk 0, then 8 "steps" (query-chunk x pair).
  - Step s runs S^T+exp for (qc,p); AV matmuls of step s-1 and the output
    projection of step s-3 ride inside its kb loop; the V projection
    (computed token-major on the PE - no DMA transposes) rides step 0 and
    remaining Q chunks ride step 1.
  - softmax denominators via the [V|1] ones column; normalization uses
    reciprocal_approx_fast + gpsimd partition-broadcast, multiplied
    straight out of PSUM into fp16 O^T.
"""

import sys

sys.path.insert(0, "/opt/trn_rl_repo")

import numpy as np

import concourse.bass as bass
import concourse.bacc as bacc
import concourse.mybir as mybir
from concourse import tile
from concourse.tile_rust import add_dep_helper
from concourse.bass_interp import get_hw_module
from concourse.bass_utils import run_bass_kernel_spmd

F16 = mybir.dt.float16
F32 = mybir.dt.float32
BF16 = mybir.dt.bfloat16

N_CORES = 8
T = 2048          # tokens per core (one batch)
D = 1024          # model dim
E = 64            # head dim
QC = 512          # query chunk
NQ = T // QC      # 4 query chunks
KB = 128          # key block
NKB = T // KB     # 16 key blocks
ND = D // 128     # 8 contraction chunks for projections

_CACHE = {}


def _build():
    nc = bacc.Bacc("TRN2", target_bir_lowering=False, debug=False,
                   num_devices=N_CORES)

    xqT = nc.dram_tensor("xqT", [D, T], F16, kind="ExternalInput").ap()
    xkT = nc.dram_tensor("xkT", [D, T], F16, kind="ExternalInput").ap()
    xvT = nc.dram_tensor("xvT", [D, T], F16, kind="ExternalInput").ap()
    # per-pair packed weights, layout [128, 8*128]: chunk d at cols d*128
    wq = [nc.dram_tensor(f"wq{p}", [128, D], F16, kind="ExternalInput").ap()
          for p in range(2)]
    wk = [nc.dram_tensor(f"wk{p}", [128, D], F16, kind="ExternalInput").ap()
          for p in range(2)]
    # all-4-head V weights for token-major projection: chunk d at cols d*256
    wv4 = nc.dram_tensor("wv4", [128, ND * 256], F16, kind="ExternalInput").ap()
    wo = [nc.dram_tensor(f"wo{p}", [128, D], F16, kind="ExternalInput").ap()
          for p in range(2)]
    pout = nc.dram_tensor("pout", [T, D], F16, kind="ExternalOutput").ap()

    with tile.TileContext(nc) as tc:
        with (
            tc.tile_pool(name="consts", bufs=1) as consts,
            tc.tile_pool(name="persist", bufs=1) as persist,
            tc.tile_pool(name="xs", bufs=1) as xs,
            tc.tile_pool(name="at", bufs=18) as atp,
            tc.tile_pool(name="o2t", bufs=2) as o2tp,
            tc.tile_pool(name="os", bufs=3) as osp,
            tc.tile_pool(name="small", bufs=2) as smallp,
            tc.tile_pool(name="psS", bufs=2, space="PSUM") as psS,
            tc.tile_pool(name="psO", bufs=1, space="PSUM") as psO,
            tc.tile_pool(name="psX", bufs=2, space="PSUM") as psX,
        ):
            # ---- weights ----
            wq_sb = [consts.tile([128, D], F16, tag=f"wq{p}", name=f"wq_sb{p}") for p in range(2)]
            wk_sb = [consts.tile([128, D], F16, tag=f"wk{p}", name=f"wk_sb{p}") for p in range(2)]
            wo_sb = [consts.tile([128, D], F16, tag=f"wo{p}", name=f"wo_sb{p}") for p in range(2)]
            wv4_sb = consts.tile([128, ND * 256], F16, tag="wv4", name="wv4_sb")
            for p in range(2):
                nc.gpsimd.dma_start(wk_sb[p][:], wk[p][:])
            nc.gpsimd.dma_start(wv4_sb[:], wv4[:])
            for p in range(2):
                nc.gpsimd.dma_start(wq_sb[p][:], wq[p][:])
                nc.gpsimd.dma_start(wo_sb[p][:], wo[p][:])

            # ---- persistent activations ----
            qt = [[persist.tile([128, QC], F16, tag=f"qt{p}_{t}", name=f"qt{p}_{t}")
                   for t in range(NQ)] for p in range(2)]
            kt = [persist.tile([128, T], F16, tag=f"kt{p}", name=f"kt{p}") for p in range(2)]
            # token(key)-major [V | 1] per (head, key-block): [128, 65] each
            v2 = [[persist.tile([128, 65], BF16, tag=f"v2_{h}_{b}", name=f"v2_{h}_{b}")
                   for b in range(NKB)] for h in range(4)]
            ones_f32 = consts.tile([128, 1], F32, tag="ones", name="ones_f32")
            nc.vector.memset(ones_f32[:], 1.0)
            for h in range(4):
                for b in range(NKB):
                    nc.vector.tensor_copy(v2[h][b][:, 64:65], ones_f32[:])

            # ---- input tiles + staged DMA issue ----
            xk = [xs.tile([128, T], F16, tag=f"xk{d}", name=f"xk{d}") for d in range(ND)]
            xq = [xs.tile([128, T], F16, tag=f"xq{d}", name=f"xq{d}") for d in range(ND)]
            xv = [xs.tile([128, T], F16, tag=f"xv{d}", name=f"xv{d}") for d in range(ND)]
            # sync queue: the critical prefixes
            for d in range(ND):
                nc.sync.dma_start(xk[d][:, 0:QC], xkT[d * 128:(d + 1) * 128, 0:QC])
            for d in range(ND):
                nc.sync.dma_start(xq[d][:, 0:QC], xqT[d * 128:(d + 1) * 128, 0:QC])
            for d in range(ND):
                nc.sync.dma_start(xv[d][:, 0:T // 2], xvT[d * 128:(d + 1) * 128, 0:T // 2])
            # gpsimd queue: the remainders
            for d in range(ND):
                nc.gpsimd.dma_start(xk[d][:, QC:T], xkT[d * 128:(d + 1) * 128, QC:T])
            for d in range(ND):
                nc.gpsimd.dma_start(xq[d][:, QC:T], xqT[d * 128:(d + 1) * 128, QC:T])
            for d in range(ND):
                nc.gpsimd.dma_start(xv[d][:, T // 2:T], xvT[d * 128:(d + 1) * 128, T // 2:T])

            # ---- helpers ----
            def proj_slice(x, w_sb_p, dst, dst_slice, pfx):
                """dst[dst_slice] = (w_sb_p^T x)[:, slice] via 8 psum-accum matmuls."""
                psk = psX.tile([128, QC], F32, tag="x", name=f"ps_{pfx}")
                for d in range(ND):
                    nc.tensor.matmul(
                        psk[:], w_sb_p[:, d * 128:(d + 1) * 128],
                        x[d][:, dst_slice], start=(d == 0), stop=(d == ND - 1))
                nc.vector.tensor_copy(dst, psk[:])

            # K projection (p-outer, t-inner) then Q chunk 0
            for p in range(2):
                for t in range(NQ):
                    sl = slice(t * QC, (t + 1) * QC)
                    proj_slice(xk, wk_sb[p], kt[p][:, sl], sl, f"k{p}_{t}")
            for p in range(2):
                proj_slice(xq, wq_sb[p], qt[p][0][:], slice(0, QC), f"q{p}_0")

            def emit_vproj(kc):
                """token-major V projection for key block kc: all 4 heads."""
                psv = psX.tile([128, 256], F32, tag="x", name=f"psv_{kc}")
                for d in range(ND):
                    nc.tensor.matmul(
                        psv[:], xv[d][:, kc * 128:(kc + 1) * 128],
                        wv4_sb[:, d * 256:(d + 1) * 256],
                        start=(d == 0), stop=(d == ND - 1))
                for h in range(4):
                    nc.vector.tensor_copy(v2[h][kc][:, 0:64],
                                          psv[:, h * 64:(h + 1) * 64])

            def emit_qproj(t):
                for p in range(2):
                    sl = slice(t * QC, (t + 1) * QC)
                    proj_slice(xq, wq_sb[p], qt[p][t][:], sl, f"q{p}_{t}")

            # ---- output projection ----
            ost_live = {}

            def emit_outproj_group(qc, o2t_pair, sub, oc, anchor):
                q0 = qc * QC
                if oc == 0:
                    ost_live[(qc, sub)] = osp.tile(
                        [128, D], F16, tag="os", name=f"os_{qc}_{sub}")
                ost = ost_live[(qc, sub)]
                pp = psX.tile([128, 512], F32, tag="x", name=f"pp_{qc}_{sub}_{oc}")
                for p in range(2):
                    mm = nc.tensor.matmul(
                        pp[:],
                        o2t_pair[p][:, sub * 128:(sub + 1) * 128],
                        wo_sb[p][:, oc * 512:(oc + 1) * 512],
                        start=(p == 0), stop=(p == 1))
                    if p == 0 and anchor is not None:
                        add_dep_helper(mm.ins, anchor.ins, sync=False,
                                       reason="interleave outproj after S")
                nc.vector.tensor_copy(ost[:, oc * 512:(oc + 1) * 512], pp[:])
                if oc == 1:
                    nc.sync.dma_start(
                        pout[q0 + sub * 128:q0 + (sub + 1) * 128, :],
                        ost[:])
                    del ost_live[(qc, sub)]

            # ---- 8-step attention pipeline ----
            steps = [(qc, p) for qc in range(NQ) for p in range(2)]
            prev = None           # (qc, p, po[2], at_list)
            o2t_all = {}          # (qc, p) -> o2t tile [128, QC] f16

            def emit_av(pr, kb):
                pqc, pp_, ppo, pat = pr
                for h in range(2):
                    nc.tensor.matmul(
                        ppo[h][:],
                        v2[2 * pp_ + h][kb][:],
                        pat[kb][:, h * QC:(h + 1) * QC],
                        start=(kb == 0), stop=(kb == NKB - 1))

            def emit_norm(pr):
                pqc, pp_, ppo, pat = pr
                o2t_t = o2tp.tile([128, QC], F16, tag=f"o2t{pp_}",
                                  name=f"o2t_{pqc}_{pp_}")
                for h in range(2):
                    r = smallp.tile([1, QC], F32, tag=f"r{h}", name=f"r_{pqc}_{pp_}_{h}")
                    nc.vector.reciprocal(r[:], ppo[h][64:65, :])
                    rb = smallp.tile([64, QC], F32, tag=f"rb{h}", name=f"rb_{pqc}_{pp_}_{h}")
                    nc.gpsimd.partition_broadcast(rb[:], r[:])
                    nc.vector.tensor_mul(
                        o2t_t[h * 64:(h + 1) * 64, :],
                        ppo[h][0:64, :], rb[:])
                o2t_all[(pqc, pp_)] = o2t_t

            for s, (qc, p) in enumerate(steps):
                po = [psO.tile([65, QC], F32, tag=f"o{h}", name=f"po_{qc}_{p}_{h}")
                      for h in range(2)]
                at_list = []
                for kb in range(NKB):
                    k0 = kb * KB
                    ps = psS.tile([128, 2 * QC], F32, tag="s", name=f"s_{qc}_{p}_{kb}")
                    s_anchor = nc.tensor.matmul(
                        ps[:, 0:QC],
                        kt[p][0:64, k0:k0 + KB],
                        qt[p][qc][0:64, :],
                        start=True, stop=True, tile_position=(0, 0))
                    nc.tensor.matmul(
                        ps[:, QC:2 * QC],
                        kt[p][64:128, k0:k0 + KB],
                        qt[p][qc][64:128, :],
                        start=True, stop=True, tile_position=(64, 0))
                    at_t = atp.tile([128, 2 * QC], BF16, tag="at", name=f"at_{qc}_{p}_{kb}")
                    nc.scalar.activation(
                        at_t[:], ps[:], mybir.ActivationFunctionType.Exp)
                    at_list.append(at_t)

                    if s == 0:
                        emit_vproj(kb)
                    if s == 1 and kb in (4, 9, 14):
                        emit_qproj({4: 1, 9: 2, 14: 3}[kb])
                    if prev is not None:
                        emit_av(prev, kb)
                    # outproj of qc'=(s-3)//2 rides odd steps >= 3
                    if s >= 3 and s % 2 == 1 and kb % 2 == 1:
                        oqc = (s - 3) // 2
                        emit_outproj_group(
                            oqc,
                            [o2t_all[(oqc, 0)], o2t_all[(oqc, 1)]],
                            kb // 4, (kb // 2) % 2, s_anchor)
                if prev is not None:
                    emit_norm(prev)
                prev = (qc, p, po, at_list)

            # ---- tail: AV + norm of the last step, outproj of qc=3 ----
            for kb in range(NKB):
                emit_av(prev, kb)
            emit_norm(prev)
            for sub in range(4):
                for oc in range(2):
                    emit_outproj_group(
                        3, [o2t_all[(3, 0)], o2t_all[(3, 1)]], sub, oc, None)

    nc.compile()
    nc.m = get_hw_module(nc.m)
    return nc


def _pack_w(w_pair):
    # w_pair: [2, 1024, 64] -> [1024, 128] -> chunk-major [128, 8*128]
    w = np.concatenate([w_pair[0], w_pair[1]], axis=1)          # [1024, 128]
    return np.ascontiguousarray(
        w.reshape(ND, 128, 128).transpose(1, 0, 2).reshape(128, D))


def _pack_wv4(w_quad):
    # w_quad: [4, 1024, 64] -> [1024, 256] -> chunk-major [128, 8*256]
    w = np.concatenate([w_quad[h] for h in range(4)], axis=1)   # [1024, 256]
    return np.ascontiguousarray(
        w.reshape(ND, 128, 256).transpose(1, 0, 2).reshape(128, ND * 256))


def _pack_wo(wo_pair):
    # wo_pair: [2, 64, 1024] -> [128, 1024]
    return np.ascontiguousarray(np.concatenate([wo_pair[0], wo_pair[1]], axis=0))


def kernel(q, k, v, W_query, W_key, W_val, W_out, _trace=False):
    q = np.asarray(q, dtype=np.float32)
    k = np.asarray(k, dtype=np.float32)
    v = np.asarray(v, dtype=np.float32)
    W_query = np.asarray(W_query, dtype=np.float32)
    W_key = np.asarray(W_key, dtype=np.float32)
    W_val = np.asarray(W_val, dtype=np.float32)
    W_out = np.asarray(W_out, dtype=np.float32)

    if "nc" not in _CACHE:
        _CACHE["nc"] = _build()
    nc = _CACHE["nc"]

    norm = 1.0 / np.sqrt(E)
    xT = {}
    for b in range(2):
        xT[("q", b)] = np.ascontiguousarray(q[b].T).astype(np.float16)
        xT[("k", b)] = np.ascontiguousarray(k[b].T).astype(np.float16)
        xT[("v", b)] = np.ascontiguousarray(v[b].T).astype(np.float16)

    in_maps = []
    for c in range(N_CORES):
        b, g = c // 4, c % 4
        hs = [4 * g, 4 * g + 1, 4 * g + 2, 4 * g + 3]
        m = {
            "xqT": xT[("q", b)], "xkT": xT[("k", b)], "xvT": xT[("v", b)],
            "wv4": _pack_wv4(W_val[hs]).astype(np.float16),
        }
        for p in range(2):
            hp = hs[2 * p:2 * p + 2]
            m[f"wq{p}"] = _pack_w(W_query[hp] * norm).astype(np.float16)
            m[f"wk{p}"] = _pack_w(W_key[hp]).astype(np.float16)
            m[f"wo{p}"] = _pack_wo(W_out[hp]).astype(np.float16)
        in_maps.append(m)

    res = run_bass_kernel_spmd(nc, in_maps, list(range(N_CORES)),
                               trace=_trace)
    parts = [res.results[c]["pout"].astype(np.float32) for c in range(N_CORES)]
    out = np.stack([
        parts[0] + parts[1] + parts[2] + parts[3],
        parts[4] + parts[5] + parts[6] + parts[7],
    ])
    if _trace:
        _CACHE["last_result"] = res
    return out


# revision 5
# speedup vs baseline: 1.4671x; 1.3451x over previous
"""Trainium2 Bass kernel for 16-head MHA (B=2, S=2048, D=1024, E=64).

Sharding: 8 cores = 2 batches x 4 head-groups. Each core computes 4 heads
(2 pairs of 2) for one batch and returns a partial output [2048, 1024]
(sum of its 4 heads' contributions after the output projection) in fp16.
Host sums the 4 partials per batch.

Per-core schedule (software-pipelined so the ACT engine, which owns the
16.8M-element exp, is saturated from ~12us):
  - K projection, then Q chunk 0, then 8 "steps" (query-chunk x pair).
  - Step s runs S^T+exp for (qc,p); AV matmuls of step s-1 and the output
    projection of step s-3 ride inside its kb loop; the V projection
    (computed token-major on the PE - no DMA transposes) rides step 0 and
    remaining Q chunks ride step 1.
  - softmax denominators via the [V|1] ones column; normalization uses
    reciprocal_approx_fast + gpsimd partition-broadcast, multiplied
    straight out of PSUM into fp16 O^T.
"""

import sys

sys.path.insert(0, "/opt/trn_rl_repo")

import numpy as np

import concourse.bass as bass
import concourse.bacc as bacc
import concourse.mybir as mybir
from concourse import tile
from concourse.tile_rust import add_dep_helper
from concourse.bass_interp import get_hw_module
from concourse.bass_utils import run_bass_kernel_spmd

F16 = mybir.dt.float16
F32 = mybir.dt.float32
BF16 = mybir.dt.bfloat16

N_CORES = 8
T = 2048          # tokens per core (one batch)
D = 1024          # model dim
E = 64            # head dim
QC = 512          # query chunk
NQ = T // QC      # 4 query chunks
KB = 128          # key block
NKB = T // KB     # 16 key blocks
ND = D // 128     # 8 contraction chunks for projections

_CACHE = {}


def _build():
    nc = bacc.Bacc("TRN2", target_bir_lowering=False, debug=False,
                   num_devices=N_CORES)

    xqT = nc.dram_tensor("xqT", [D, T], F16, kind="ExternalInput").ap()
    xkT = nc.dram_tensor("xkT", [D, T], F16, kind="ExternalInput").ap()
    xvT = nc.dram_tensor("xvT", [D, T], F16, kind="ExternalInput").ap()
    # per-pair packed weights, layout [128, 8*128]: chunk d at cols d*128
    wq = [nc.dram_tensor(f"wq{p}", [128, D], F16, kind="ExternalInput").ap()
          for p in range(2)]
    wk = [nc.dram_tensor(f"wk{p}", [128, D], F16, kind="ExternalInput").ap()
          for p in range(2)]
    # all-4-head V weights for token-major projection: chunk d at cols d*256
    wv4 = nc.dram_tensor("wv4", [128, ND * 256], F16, kind="ExternalInput").ap()
    wo = [nc.dram_tensor(f"wo{p}", [128, D], F16, kind="ExternalInput").ap()
          for p in range(2)]
    pout = nc.dram_tensor("pout", [T, D], F16, kind="ExternalOutput").ap()

    with tile.TileContext(nc) as tc:
        with (
            tc.tile_pool(name="consts", bufs=1) as consts,
            tc.tile_pool(name="persist", bufs=1) as persist,
            tc.tile_pool(name="xs", bufs=1) as xs,
            tc.tile_pool(name="at", bufs=18) as atp,
            tc.tile_pool(name="o2t", bufs=2) as o2tp,
            tc.tile_pool(name="os", bufs=3) as osp,
            tc.tile_pool(name="small", bufs=1) as smallp,
            tc.tile_pool(name="psS", bufs=2, space="PSUM") as psS,
            tc.tile_pool(name="psO", bufs=1, space="PSUM") as psO,
            tc.tile_pool(name="psX", bufs=2, space="PSUM") as psX,
        ):
            # ---- weights ----
            wq_sb = [consts.tile([128, D], F16, tag=f"wq{p}", name=f"wq_sb{p}") for p in range(2)]
            wk_sb = [consts.tile([128, D], F16, tag=f"wk{p}", name=f"wk_sb{p}") for p in range(2)]
            wo_sb = [consts.tile([128, D], F16, tag=f"wo{p}", name=f"wo_sb{p}") for p in range(2)]
            wv4_sb = consts.tile([128, ND * 256], F16, tag="wv4", name="wv4_sb")
            for p in range(2):
                nc.gpsimd.dma_start(wk_sb[p][:], wk[p][:])
            nc.gpsimd.dma_start(wv4_sb[:], wv4[:])
            for p in range(2):
                nc.gpsimd.dma_start(wq_sb[p][:], wq[p][:])
                nc.gpsimd.dma_start(wo_sb[p][:], wo[p][:])

            # ---- persistent activations ----
            qt = [[persist.tile([128, QC], F16, tag=f"qt{p}_{t}", name=f"qt{p}_{t}")
                   for t in range(NQ)] for p in range(2)]
            kt = [persist.tile([128, T], F16, tag=f"kt{p}", name=f"kt{p}") for p in range(2)]
            # token(key)-major [V | 1] per (head, key-block): [128, 65] each
            v2 = [[persist.tile([128, 65], BF16, tag=f"v2_{h}_{b}", name=f"v2_{h}_{b}")
                   for b in range(NKB)] for h in range(4)]
            ones_f32 = consts.tile([128, 1], F32, tag="ones", name="ones_f32")
            nc.vector.memset(ones_f32[:], 1.0)
            for h in range(4):
                for b in range(NKB):
                    nc.vector.tensor_copy(v2[h][b][:, 64:65], ones_f32[:])

            # ---- input tiles + staged DMA issue ----
            xk = [xs.tile([128, T], F16, tag=f"xk{d}", name=f"xk{d}") for d in range(ND)]
            xq = [xs.tile([128, T], F16, tag=f"xq{d}", name=f"xq{d}") for d in range(ND)]
            xv = [xs.tile([128, T], F16, tag=f"xv{d}", name=f"xv{d}") for d in range(ND)]
            # sync queue: the critical prefixes
            for d in range(ND):
                nc.sync.dma_start(xk[d][:, 0:QC], xkT[d * 128:(d + 1) * 128, 0:QC])
            for d in range(ND):
                nc.sync.dma_start(xq[d][:, 0:QC], xqT[d * 128:(d + 1) * 128, 0:QC])
            for d in range(ND):
                nc.sync.dma_start(xv[d][:, 0:T // 2], xvT[d * 128:(d + 1) * 128, 0:T // 2])
            # gpsimd queue: the remainders
            for d in range(ND):
                nc.gpsimd.dma_start(xk[d][:, QC:T], xkT[d * 128:(d + 1) * 128, QC:T])
            for d in range(ND):
                nc.gpsimd.dma_start(xq[d][:, QC:T], xqT[d * 128:(d + 1) * 128, QC:T])
            for d in range(ND):
                nc.gpsimd.dma_start(xv[d][:, T // 2:T], xvT[d * 128:(d + 1) * 128, T // 2:T])

            # ---- helpers ----
            def proj_slice(x, w_sb_p, dst, dst_slice, pfx):
                """dst[dst_slice] = (w_sb_p^T x)[:, slice] via 8 psum-accum matmuls."""
                psk = psX.tile([128, QC], F32, tag="x", name=f"ps_{pfx}")
                for d in range(ND):
                    nc.tensor.matmul(
                        psk[:], w_sb_p[:, d * 128:(d + 1) * 128],
                        x[d][:, dst_slice], start=(d == 0), stop=(d == ND - 1))
                nc.vector.tensor_copy(dst, psk[:])

            # K projection (p-outer, t-inner) then Q chunk 0
            for p in range(2):
                for t in range(NQ):
                    sl = slice(t * QC, (t + 1) * QC)
                    proj_slice(xk, wk_sb[p], kt[p][:, sl], sl, f"k{p}_{t}")
            for p in range(2):
                proj_slice(xq, wq_sb[p], qt[p][0][:], slice(0, QC), f"q{p}_0")

            def emit_vproj(kc):
                """token-major V projection for key block kc: all 4 heads."""
                psv = psX.tile([128, 256], F32, tag="x", name=f"psv_{kc}")
                for d in range(ND):
                    nc.tensor.matmul(
                        psv[:], xv[d][:, kc * 128:(kc + 1) * 128],
                        wv4_sb[:, d * 256:(d + 1) * 256],
                        start=(d == 0), stop=(d == ND - 1))
                for h in range(4):
                    nc.vector.tensor_copy(v2[h][kc][:, 0:64],
                                          psv[:, h * 64:(h + 1) * 64])

            def emit_qproj(t):
                for p in range(2):
                    sl = slice(t * QC, (t + 1) * QC)
                    proj_slice(xq, wq_sb[p], qt[p][t][:], sl, f"q{p}_{t}")

            # ---- output projection ----
            ost_live = {}

            def emit_outproj_group(qc, o2t_pair, sub, oc, anchor):
                q0 = qc * QC
                if oc == 0:
                    ost_live[(qc, sub)] = osp.tile(
                        [128, D], F16, tag="os", name=f"os_{qc}_{sub}")
                ost = ost_live[(qc, sub)]
                pp = psX.tile([128, 512], F32, tag="x", name=f"pp_{qc}_{sub}_{oc}")
                for p in range(2):
                    mm = nc.tensor.matmul(
                        pp[:],
                        o2t_pair[p][:, sub * 128:(sub + 1) * 128],
                        wo_sb[p][:, oc * 512:(oc + 1) * 512],
                        start=(p == 0), stop=(p == 1))
                    if p == 0 and anchor is not None:
                        add_dep_helper(mm.ins, anchor.ins, sync=False,
                                       reason="interleave outproj after S")
                nc.vector.tensor_copy(ost[:, oc * 512:(oc + 1) * 512], pp[:])
                if oc == 1:
                    nc.sync.dma_start(
                        pout[q0 + sub * 128:q0 + (sub + 1) * 128, :],
                        ost[:])
                    del ost_live[(qc, sub)]

            # ---- 8-step attention pipeline ----
            steps = [(qc, p) for qc in range(NQ) for p in range(2)]
            prev = None           # (qc, p, po[2], at_list)
            o2t_all = {}          # (qc, p) -> o2t tile [128, QC] f16

            def emit_av(pr, kb):
                pqc, pp_, ppo, pat = pr
                for h in range(2):
                    nc.tensor.matmul(
                        ppo[h][:],
                        v2[2 * pp_ + h][kb][:],
                        pat[kb][:, h * QC:(h + 1) * QC],
                        start=(kb == 0), stop=(kb == NKB - 1))

            def emit_norm(pr):
                pqc, pp_, ppo, pat = pr
                o2t_t = o2tp.tile([128, QC], F16, tag=f"o2t{pp_}",
                                  name=f"o2t_{pqc}_{pp_}")
                for h in range(2):
                    d_sb = smallp.tile([1, QC], F32, tag=f"d{h}", name=f"d_{pqc}_{pp_}_{h}")
                    nc.vector.tensor_copy(d_sb[:], ppo[h][64:65, :])
                    r = smallp.tile([1, QC], F32, tag=f"r{h}", name=f"r_{pqc}_{pp_}_{h}")
                    nc.vector.reciprocal_approx_fast(r[:], d_sb[:])
                    rb = smallp.tile([64, QC], F32, tag=f"rb{h}", name=f"rb_{pqc}_{pp_}_{h}")
                    nc.gpsimd.partition_broadcast(rb[:], r[:])
                    nc.vector.tensor_mul(
                        o2t_t[h * 64:(h + 1) * 64, :],
                        ppo[h][0:64, :], rb[:])
                o2t_all[(pqc, pp_)] = o2t_t

            for s, (qc, p) in enumerate(steps):
                po = [psO.tile([65, QC], F32, tag=f"o{h}", name=f"po_{qc}_{p}_{h}")
                      for h in range(2)]
                at_list = []
                for kb in range(NKB):
                    k0 = kb * KB
                    ps = psS.tile([128, 2 * QC], F32, tag="s", name=f"s_{qc}_{p}_{kb}")
                    s_anchor = nc.tensor.matmul(
                        ps[:, 0:QC],
                        kt[p][0:64, k0:k0 + KB],
                        qt[p][qc][0:64, :],
                        start=True, stop=True, tile_position=(0, 0))
                    nc.tensor.matmul(
                        ps[:, QC:2 * QC],
                        kt[p][64:128, k0:k0 + KB],
                        qt[p][qc][64:128, :],
                        start=True, stop=True, tile_position=(64, 0))
                    at_t = atp.tile([128, 2 * QC], BF16, tag="at", name=f"at_{qc}_{p}_{kb}")
                    nc.scalar.activation(
                        at_t[:], ps[:], mybir.ActivationFunctionType.Exp)
                    at_list.append(at_t)

                    if s == 0:
                        emit_vproj(kb)
                    if s == 1 and kb in (4, 9, 14):
                        emit_qproj({4: 1, 9: 2, 14: 3}[kb])
                    if prev is not None:
                        emit_av(prev, kb)
                    # outproj of qc'=(s-3)//2 rides odd steps >= 3
                    if s >= 3 and s % 2 == 1 and kb % 2 == 1:
                        oqc = (s - 3) // 2
                        emit_outproj_group(
                            oqc,
                            [o2t_all[(oqc, 0)], o2t_all[(oqc, 1)]],
                            kb // 4, (kb // 2) % 2, s_anchor)
                if prev is not None:
                    emit_norm(prev)
                prev = (qc, p, po, at_list)

            # ---- tail: AV + norm of the last step, outproj of qc=3 ----
            for kb in range(NKB):
                emit_av(prev, kb)
            emit_norm(prev)
            for sub in range(4):
                for oc in range(2):
                    emit_outproj_group(
                        3, [o2t_all[(3, 0)], o2t_all[(3, 1)]], sub, oc, None)

    nc.compile()
    nc.m = get_hw_module(nc.m)
    return nc


def _pack_w(w_pair):
    # w_pair: [2, 1024, 64] -> [1024, 128] -> chunk-major [128, 8*128]
    w = np.concatenate([w_pair[0], w_pair[1]], axis=1)          # [1024, 128]
    return np.ascontiguousarray(
        w.reshape(ND, 128, 128).transpose(1, 0, 2).reshape(128, D))


def _pack_wv4(w_quad):
    # w_quad: [4, 1024, 64] -> [1024, 256] -> chunk-major [128, 8*256]
    w = np.concatenate([w_quad[h] for h in range(4)], axis=1)   # [1024, 256]
    return np.ascontiguousarray(
        w.reshape(ND, 128, 256).transpose(1, 0, 2).reshape(128, ND * 256))


def _pack_wo(wo_pair):
    # wo_pair: [2, 64, 1024] -> [128, 1024]
    return np.ascontiguousarray(np.concatenate([wo_pair[0], wo_pair[1]], axis=0))


def kernel(q, k, v, W_query, W_key, W_val, W_out, _trace=False):
    q = np.asarray(q, dtype=np.float32)
    k = np.asarray(k, dtype=np.float32)
    v = np.asarray(v, dtype=np.float32)
    W_query = np.asarray(W_query, dtype=np.float32)
    W_key = np.asarray(W_key, dtype=np.float32)
    W_val = np.asarray(W_val, dtype=np.float32)
    W_out = np.asarray(W_out, dtype=np.float32)

    if "nc" not in _CACHE:
        _CACHE["nc"] = _build()
    nc = _CACHE["nc"]

    norm = 1.0 / np.sqrt(E)
    xT = {}
    for b in range(2):
        xT[("q", b)] = np.ascontiguousarray(q[b].T).astype(np.float16)
        xT[("k", b)] = np.ascontiguousarray(k[b].T).astype(np.float16)
        xT[("v", b)] = np.ascontiguousarray(v[b].T).astype(np.float16)

    in_maps = []
    for c in range(N_CORES):
        b, g = c // 4, c % 4
        hs = [4 * g, 4 * g + 1, 4 * g + 2, 4 * g + 3]
        m = {
            "xqT": xT[("q", b)], "xkT": xT[("k", b)], "xvT": xT[("v", b)],
            "wv4": _pack_wv4(W_val[hs]).astype(np.float16),
        }
        for p in range(2):
            hp = hs[2 * p:2 * p + 2]
            m[f"wq{p}"] = _pack_w(W_query[hp] * norm).astype(np.float16)
            m[f"wk{p}"] = _pack_w(W_key[hp]).astype(np.float16)
            m[f"wo{p}"] = _pack_wo(W_out[hp]).astype(np.float16)
        in_maps.append(m)

    res = run_bass_kernel_spmd(nc, in_maps, list(range(N_CORES)),
                               trace=_trace)
    parts = [res.results[c]["pout"].astype(np.float32) for c in range(N_CORES)]
    out = np.stack([
        parts[0] + parts[1] + parts[2] + parts[3],
        parts[4] + parts[5] + parts[6] + parts[7],
    ])
    if _trace:
        _CACHE["last_result"] = res
    return out
